# revision 2
# baseline (speedup 1.0000x reference)
"""LightGCN 3-layer propagation on 8 Trainium2 NeuronCores (Bass/Tile).

Strategy (dest-row sharding, per spec sharding_hint):
  - Node table padded 150000 -> 150528 rows; core c owns output rows
    [18816*c, 18816*(c+1)) = 147 tiles of 128 rows.
  - Edges partitioned by destination row; per (core, dest-tile) the edge
    list is padded to a uniform Q=8 blocks of 128 edges (pad: idx=0,val=0)
    so all 8 cores run the IDENTICAL program (SPMD), only data differs.
  - Per block: indirect-DMA gather of 128 source rows E[col] -> SBUF,
    DVE builds W[p,dest] = (iota==ldest[p])*val[p] in one fused op,
    PE matmul accumulates psum[dest,:] += W.T @ gathered  (segment sum).
  - Per tile: ACT copies psum -> new-E tile (staged to HBM for AllGather),
    DVE adds psum into the running layer-sum accumulator.
  - AllGather of the 147-tile shard between layers (layers 0,1 only).
  - Output = (E0 + E1 + E2 + E3)/4 for the core's shard; host concatenates.
"""
import sys
import numpy as np

sys.path.insert(0, "/opt/trn_rl_repo")

NUM_USERS = 100000
NUM_ITEMS = 50000
N_NODES = 150000
N_PAD = 150528          # 8 * 18816
SHARD = 18816           # 147 tiles of 128
T = 147
P = 128
D = 64
Q = 8                   # blocks (of 128 edge slots) per dest tile; max real is 955
LAYERS = 3
N_CORES = 8

_CACHE = {}


def _build_module():
    import concourse.bass as bass
    import concourse.bacc as bacc
    import concourse.tile as tile
    import concourse.mybir as mybir

    nc = bacc.Bacc("TRN2", target_bir_lowering=False, debug=False,
                   num_devices=N_CORES)
    NB = T * Q  # 1176 blocks
    t_e0 = nc.dram_tensor("e0", [N_PAD, D], mybir.dt.float32, kind="ExternalInput")
    t_e0s = nc.dram_tensor("e0s", [SHARD, D], mybir.dt.float32, kind="ExternalInput")
    t_idx = nc.dram_tensor("idx", [P, NB], mybir.dt.int32, kind="ExternalInput")
    t_val = nc.dram_tensor("val", [P, NB], mybir.dt.float32, kind="ExternalInput")
    t_ld = nc.dram_tensor("ld", [P, NB], mybir.dt.float32, kind="ExternalInput")
    t_iota = nc.dram_tensor("iota", [P, P], mybir.dt.float32, kind="ExternalInput")
    t_out = nc.dram_tensor("out", [SHARD, D], mybir.dt.float32, kind="ExternalOutput")

    with tile.TileContext(nc) as tc:
        with (
            tc.tile_pool(name="stat", bufs=1) as stat,
            tc.tile_pool(name="gp", bufs=12) as gp,
            tc.tile_pool(name="wp", bufs=8) as wp,
            tc.tile_pool(name="ep", bufs=6) as ep,
            tc.tile_pool(name="psum", bufs=6, space="PSUM") as pp,
            tc.tile_pool(name="dram", bufs=1, space="DRAM") as dram,
        ):
            iota = stat.tile([P, P], mybir.dt.float32)
            nc.sync.dma_start(out=iota[:], in_=t_iota[:])
            idx_sb = stat.tile([P, NB], mybir.dt.int32)
            val_sb = stat.tile([P, NB], mybir.dt.float32)
            ld_sb = stat.tile([P, NB], mybir.dt.float32)
            nc.sync.dma_start(out=idx_sb[:], in_=t_idx[:])
            nc.sync.dma_start(out=val_sb[:], in_=t_val[:])
            nc.sync.dma_start(out=ld_sb[:], in_=t_ld[:])

            # running sum over layers, [128, T*64]; init with E0 shard
            accout = stat.tile([P, T * D], mybir.dt.float32)
            nc.sync.dma_start(
                out=accout[:].rearrange("p (t d) -> p t d", d=D),
                in_=t_e0s[:].rearrange("(t p) d -> p t d", p=P),
            )

            ag_in = [dram.tile([SHARD, D], mybir.dt.float32,
                               name=f"agi{k}", tag=f"agi{k}")
                     for k in range(LAYERS - 1)]
            ag_out = [dram.tile([N_PAD, D], mybir.dt.float32,
                                name=f"ago{k}", tag=f"ago{k}")
                      for k in range(LAYERS - 1)]

            for layer in range(LAYERS):
                src = t_e0 if layer == 0 else ag_out[layer - 1]
                for t in range(T):
                    ps = pp.tile([P, D], mybir.dt.float32, space="PSUM", tag="ps")
                    for q in range(Q):
                        b = t * Q + q
                        g = gp.tile([P, D], mybir.dt.float32, tag="g")
                        nc.gpsimd.indirect_dma_start(
                            out=g[:], out_offset=None, in_=src[:],
                            in_offset=bass.IndirectOffsetOnAxis(
                                ap=idx_sb[:, b:b + 1], axis=0),
                        )
                        w = wp.tile([P, P], mybir.dt.float32, tag="w")
                        nc.vector.tensor_scalar(
                            out=w[:], in0=iota[:],
                            scalar1=ld_sb[:, b:b + 1],
                            scalar2=val_sb[:, b:b + 1],
                            op0=mybir.AluOpType.is_equal,
                            op1=mybir.AluOpType.mult,
                        )
                        nc.tensor.matmul(out=ps[:], lhsT=w[:], rhs=g[:],
                                         start=(q == 0), stop=(q == Q - 1))
                    # accumulate layer output into running sum (DVE)
                    nc.vector.tensor_add(
                        out=accout[:, t * D:(t + 1) * D],
                        in0=accout[:, t * D:(t + 1) * D],
                        in1=ps[:],
                    )
                    if layer < LAYERS - 1:
                        # stage new-E tile for AllGather (ACT copies psum->sbuf)
                        en = ep.tile([P, D], mybir.dt.float32, tag="en")
                        nc.scalar.copy(out=en[:], in_=ps[:])
                        nc.sync.dma_start(
                            out=ag_in[layer][t * P:(t + 1) * P, :], in_=en[:])
                if layer < LAYERS - 1:
                    nc.gpsimd.collective_compute(
                        "AllGather", mybir.AluOpType.bypass,
                        replica_groups=[list(range(N_CORES))],
                        ins=[ag_in[layer].opt()],
                        outs=[ag_out[layer].opt()],
                    )

            # write out accout/4
            for t in range(T):
                ob = ep.tile([P, D], mybir.dt.float32, tag="ob")
                nc.scalar.mul(out=ob[:], in_=accout[:, t * D:(t + 1) * D], mul=0.25)
                nc.sync.dma_start(out=t_out[t * P:(t + 1) * P, :], in_=ob[:])

    nc.compile()
    return nc


def _prep_inputs(user_emb_w, item_emb_w, edge_row, edge_col, edge_val):
    e_full = np.concatenate([np.asarray(user_emb_w, np.float32),
                             np.asarray(item_emb_w, np.float32)], axis=0)
    e0 = np.zeros((N_PAD, D), np.float32)
    e0[:N_NODES] = e_full
    row = np.asarray(edge_row, np.int64)
    col = np.asarray(edge_col, np.int64)
    val = np.asarray(edge_val, np.float32)

    core_of = row // SHARD
    in_maps = []
    iota_np = np.tile(np.arange(P, dtype=np.float32), (P, 1))
    NB = T * Q
    for c in range(N_CORES):
        m = core_of == c
        r, cl, v = row[m] - c * SHARD, col[m], val[m]
        t_arr = r // P
        order = np.lexsort((cl, t_arr))
        r, cl, v, t_arr = r[order], cl[order], v[order], t_arr[order]
        idx_a = np.zeros((NB, P), np.int32)
        val_a = np.zeros((NB, P), np.float32)
        ld_a = np.zeros((NB, P), np.float32)
        # slot edges per tile
        start = np.searchsorted(t_arr, np.arange(T))
        end = np.searchsorted(t_arr, np.arange(T) + 1)
        for t in range(T):
            n = end[t] - start[t]
            if n > Q * P:
                raise RuntimeError(f"tile overflow {n} > {Q*P}")
            sl = slice(start[t], end[t])
            flat_idx = np.zeros(Q * P, np.int32)
            flat_val = np.zeros(Q * P, np.float32)
            flat_ld = np.zeros(Q * P, np.float32)
            flat_idx[:n] = cl[sl]
            flat_val[:n] = v[sl]
            flat_ld[:n] = (r[sl] % P).astype(np.float32)
            idx_a[t * Q:(t + 1) * Q] = flat_idx.reshape(Q, P)
            val_a[t * Q:(t + 1) * Q] = flat_val.reshape(Q, P)
            ld_a[t * Q:(t + 1) * Q] = flat_ld.reshape(Q, P)
        in_maps.append({
            "e0": e0,
            "e0s": e0[c * SHARD:(c + 1) * SHARD].copy(),
            "idx": idx_a.T.copy(),
            "val": val_a.T.copy(),
            "ld": ld_a.T.copy(),
            "iota": iota_np,
        })
    return in_maps


def kernel(user_emb_w, item_emb_w, edge_row, edge_col, edge_val):
    from concourse import bass_utils
    if "nc" not in _CACHE:
        _CACHE["nc"] = _build_module()
    nc = _CACHE["nc"]
    in_maps = _prep_inputs(user_emb_w, item_emb_w, edge_row, edge_col, edge_val)
    res = bass_utils.run_bass_kernel_spmd(nc, in_maps, core_ids=list(range(N_CORES)))
    full = np.concatenate([res.results[c]["out"] for c in range(N_CORES)], axis=0)
    full = full[:N_NODES]
    return full[:NUM_USERS], full[NUM_USERS:]


# revision 7
# speedup vs baseline: 2.9240x; 2.9240x over previous
"""LightGCN 3-layer propagation on 8 Trainium2 NeuronCores (Bass/Tile).

Strategy (dest-row sharding, per spec sharding_hint):
  - Node table padded 150000 -> 150528 rows; core c owns output rows
    [18816*c, 18816*(c+1)) = 147 tiles of 128 rows.
  - Edges partitioned by destination row; per (core, dest-tile) the edge
    list is padded to a uniform Q=8 blocks of 128 edges (pad: idx=0,val=0)
    so all 8 cores run the IDENTICAL program (SPMD), only data differs.
  - Per block: indirect-DMA gather of 128 source rows E[col] -> SBUF,
    DVE builds W[p,dest] = (iota==ldest[p])*val[p] in one fused op,
    PE matmul accumulates psum[dest,:] += W.T @ gathered  (segment sum).
  - Per tile: ACT copies psum -> new-E tile (staged to HBM for AllGather),
    DVE adds psum into the running layer-sum accumulator.
  - AllGather of the 147-tile shard between layers (layers 0,1 only).
  - Output = (E0 + E1 + E2 + E3)/4 for the core's shard; host concatenates.
"""
import sys
import numpy as np

sys.path.insert(0, "/opt/trn_rl_repo")

NUM_USERS = 100000
NUM_ITEMS = 50000
N_NODES = 150000
N_PAD = 150528          # 8 * 18816
SHARD = 18816           # 147 tiles of 128
T = 147
P = 128
D = 64
Q = 8                   # blocks (of 128 edge slots) per dest tile; max real is 955
LAYERS = 3
N_CORES = 8

_CACHE = {}


def _build_module():
    import concourse.bass as bass
    import concourse.bacc as bacc
    import concourse.tile as tile
    import concourse.mybir as mybir

    nc = bacc.Bacc("TRN2", target_bir_lowering=False, debug=False,
                   num_devices=N_CORES)
    NB = T * Q  # 1176 blocks
    t_e0s = nc.dram_tensor("e0s", [SHARD, D], mybir.dt.float32, kind="ExternalInput")
    t_idx = nc.dram_tensor("idx", [P, NB], mybir.dt.int32, kind="ExternalInput")
    t_val = nc.dram_tensor("val", [P, NB], mybir.dt.float32, kind="ExternalInput")
    t_ld = nc.dram_tensor("ld", [P, NB], mybir.dt.float32, kind="ExternalInput")
    t_iota = nc.dram_tensor("iota", [P, P], mybir.dt.float32, kind="ExternalInput")
    t_out = nc.dram_tensor("out", [SHARD, D], mybir.dt.float32, kind="ExternalOutput")

    with tile.TileContext(nc) as tc:
        with (
            tc.tile_pool(name="stat", bufs=1) as stat,
            tc.tile_pool(name="gp", bufs=12) as gp,
            tc.tile_pool(name="wp", bufs=8) as wp,
            tc.tile_pool(name="ep", bufs=6) as ep,
            tc.tile_pool(name="psum", bufs=6, space="PSUM") as pp,
            tc.tile_pool(name="dram", bufs=1, space="DRAM") as dram,
        ):
            iota = stat.tile([P, P], mybir.dt.float32)
            nc.sync.dma_start(out=iota[:], in_=t_iota[:])
            idx_sb = stat.tile([P, NB], mybir.dt.int32)
            val_sb = stat.tile([P, NB], mybir.dt.float32)
            ld_sb = stat.tile([P, NB], mybir.dt.float32)
            nc.sync.dma_start(out=idx_sb[:], in_=t_idx[:])
            nc.sync.dma_start(out=val_sb[:], in_=t_val[:])
            nc.sync.dma_start(out=ld_sb[:], in_=t_ld[:])

            # running sum over layers, [128, T*64]; init with E0 shard
            accout = stat.tile([P, T * D], mybir.dt.float32)
            nc.sync.dma_start(
                out=accout[:].rearrange("p (t d) -> p t d", d=D),
                in_=t_e0s[:].rearrange("(t p) d -> p t d", p=P),
            )

            ag_in = [dram.tile([SHARD, D], mybir.dt.float32,
                               name=f"agi{k}", tag=f"agi{k}")
                     for k in range(LAYERS)]
            ag_out = [dram.tile([N_PAD, D], mybir.dt.float32,
                                name=f"ago{k}", tag=f"ago{k}")
                      for k in range(LAYERS)]

            # all-gather E0 shards into the full table (avoids shipping the
            # full table to every core from the host); stage via SBUF —
            # DRAM->DRAM DMA is not supported on this path
            for t in range(T):
                eb = ep.tile([P, D], mybir.dt.float32, tag="en")
                nc.vector.tensor_copy(
                    out=eb[:],
                    in_=accout[:, t * D:(t + 1) * D])
                nc.sync.dma_start(out=ag_in[0][t * P:(t + 1) * P, :], in_=eb[:])
            nc.gpsimd.collective_compute(
                "AllGather", mybir.AluOpType.bypass,
                replica_groups=[list(range(N_CORES))],
                ins=[ag_in[0].opt()],
                outs=[ag_out[0].opt()],
            )

            for layer in range(LAYERS):
                src = ag_out[layer]
                for t in range(T):
                    ps = pp.tile([P, D], mybir.dt.float32, space="PSUM", tag="ps")
                    for q in range(Q):
                        b = t * Q + q
                        g = gp.tile([P, D], mybir.dt.float32, tag="g")
                        nc.gpsimd.indirect_dma_start(
                            out=g[:], out_offset=None, in_=src[:],
                            in_offset=bass.IndirectOffsetOnAxis(
                                ap=idx_sb[:, b:b + 1], axis=0),
                        )
                        w = wp.tile([P, P], mybir.dt.float32, tag="w")
                        nc.vector.tensor_scalar(
                            out=w[:], in0=iota[:],
                            scalar1=ld_sb[:, b:b + 1],
                            scalar2=val_sb[:, b:b + 1],
                            op0=mybir.AluOpType.is_equal,
                            op1=mybir.AluOpType.mult,
                        )
                        nc.tensor.matmul(out=ps[:], lhsT=w[:], rhs=g[:],
                                         start=(q == 0), stop=(q == Q - 1))
                    # accumulate layer output into running sum (DVE)
                    nc.vector.tensor_add(
                        out=accout[:, t * D:(t + 1) * D],
                        in0=accout[:, t * D:(t + 1) * D],
                        in1=ps[:],
                    )
                    if layer < LAYERS - 1:
                        # stage new-E tile for AllGather (ACT copies psum->sbuf)
                        en = ep.tile([P, D], mybir.dt.float32, tag="en")
                        nc.scalar.copy(out=en[:], in_=ps[:])
                        nc.sync.dma_start(
                            out=ag_in[layer + 1][t * P:(t + 1) * P, :], in_=en[:])
                if layer < LAYERS - 1:
                    nc.gpsimd.collective_compute(
                        "AllGather", mybir.AluOpType.bypass,
                        replica_groups=[list(range(N_CORES))],
                        ins=[ag_in[layer + 1].opt()],
                        outs=[ag_out[layer + 1].opt()],
                    )

            # write out accout/4
            for t in range(T):
                ob = ep.tile([P, D], mybir.dt.float32, tag="ob")
                nc.scalar.mul(out=ob[:], in_=accout[:, t * D:(t + 1) * D], mul=0.25)
                nc.sync.dma_start(out=t_out[t * P:(t + 1) * P, :], in_=ob[:])

    nc.compile()
    return nc


def _prep_inputs(user_emb_w, item_emb_w, edge_row, edge_col, edge_val):
    e_full = np.concatenate([np.asarray(user_emb_w, np.float32),
                             np.asarray(item_emb_w, np.float32)], axis=0)
    e0 = np.zeros((N_PAD, D), np.float32)
    e0[:N_NODES] = e_full
    row = np.asarray(edge_row, np.int64)
    col = np.asarray(edge_col, np.int64)
    val = np.asarray(edge_val, np.float32)

    # global slot assignment, fully vectorized:
    #   global tile g = row // 128 in [0, 1176); core = g // T
    #   edge rank within its tile -> slot = g*Q*128 + rank
    iota_np = np.tile(np.arange(P, dtype=np.float32), (P, 1))
    NB = T * Q
    GT = N_CORES * T
    g = row // P
    order = np.lexsort((col, g))
    gs, cls, vs, rs = g[order], col[order], val[order], (row % P)[order]
    counts = np.bincount(gs, minlength=GT)
    if counts.max() > Q * P:
        raise RuntimeError(f"tile overflow {counts.max()} > {Q*P}")
    starts = np.concatenate([[0], np.cumsum(counts)[:-1]])
    rank = np.arange(len(gs)) - np.repeat(starts, counts)
    slot = gs * (Q * P) + rank
    idx_f = np.zeros(GT * Q * P, np.int32)
    val_f = np.zeros(GT * Q * P, np.float32)
    ld_f = np.zeros(GT * Q * P, np.float32)
    idx_f[slot] = cls
    val_f[slot] = vs
    ld_f[slot] = rs
    # per core: [NB, P] -> transpose to [P, NB] SBUF layout
    idx_f = idx_f.reshape(N_CORES, NB, P)
    val_f = val_f.reshape(N_CORES, NB, P)
    ld_f = ld_f.reshape(N_CORES, NB, P)
    in_maps = []
    for c in range(N_CORES):
        in_maps.append({
            "e0s": e0[c * SHARD:(c + 1) * SHARD].copy(),
            "idx": np.ascontiguousarray(idx_f[c].T),
            "val": np.ascontiguousarray(val_f[c].T),
            "ld": np.ascontiguousarray(ld_f[c].T),
            "iota": iota_np,
        })
    return in_maps


def kernel(user_emb_w, item_emb_w, edge_row, edge_col, edge_val):
    from concourse import bass_utils
    if "nc" not in _CACHE:
        _CACHE["nc"] = _build_module()
    nc = _CACHE["nc"]
    in_maps = _prep_inputs(user_emb_w, item_emb_w, edge_row, edge_col, edge_val)
    res = bass_utils.run_bass_kernel_spmd(nc, in_maps, core_ids=list(range(N_CORES)))
    full = np.concatenate([res.results[c]["out"] for c in range(N_CORES)], axis=0)
    full = full[:N_NODES]
    return full[:NUM_USERS], full[NUM_USERS:]
